# revision 12
# baseline (speedup 1.0000x reference)
"""Trainium2 Bass kernel for ContentSelectionCell.

Computes, for full inputs x[64,512], enc_outs[64,2048,512], W[1024,512], b[512],
actual_step scalar:

    scores  = einsum('bd,btd->bt', x, enc_outs); scores[:, step] = -1e9
    align   = softmax(scores, -1)
    context = einsum('bt,btd->bd', align, enc_outs)
    att     = sigmoid(concat([x, context], -1) @ W + b)
    out     = att * x

Sharding: data-parallel over batch, 8 batches per core on 8 NeuronCores.
Per-core dataflow (single pass over enc, which is the 256 MB memory roofline):
  - enc[b] is loaded as [128p, 16k, 512d] tiles (t = 16*p + k).
  - scores via fused DVE tensor_tensor_reduce (mul + free-dim reduce), with the
    step mask folded in as the reduction's init value.
  - softmax max/sum cross-partition steps via tiny PE transpose / ones-matmuls.
  - context accumulated on PE: 16 matmuls [K=128t, M=1, N=512d] into PSUM.
  - final Dense runs batched over all 8 local rows with host-pretransposed
    x^T / W-chunk layouts; bias folded in as a K=1 ones-matmul.
"""

import os
from contextlib import ExitStack

import numpy as np

import concourse.bacc as bacc
import concourse.bass as bass
import concourse.tile as tile
from concourse import mybir
from concourse.bass_utils import run_bass_kernel_spmd
from concourse.masks import make_identity

N_CORES = 8
B, T, D, H = 64, 2048, 512, 512
BL = B // N_CORES  # local batches per core
KCH = 16           # T chunks per batch: t = 16*p + k
NEG = -1e9

F32 = mybir.dt.float32
AO = mybir.AluOpType
AF = mybir.ActivationFunctionType

# mm_dtype for the heavy PE matmuls (context accumulation + dense):
#   float32  -> exact, 4 cycles/row
#   float32r -> 1 cycle/row at N>=256, reduced multiply precision
MM_DTYPE = os.environ.get("CSEL_MM_DTYPE", "float32")

_CACHE = {}


def _ensure_ntff_hook():
    """Register the axon NTFF profiling hook if the image's antenv lacks it.

    Needed only for trace=True runs (HW exec-time measurement); execution
    works without it. Best-effort: failures silently degrade to no-trace.
    """
    import sys
    import types

    try:
        from antenv.axon_hooks import get_axon_ntff_profile_hook  # noqa: F401

        return
    except ImportError:
        pass
    try:
        import antenv
        from trn_agent_boot.trn_boot import _ntff_profile_via_ctypes

        hook = _ntff_profile_via_ctypes("/opt/axon/libaxon_pjrt.so")
        mod = types.ModuleType("antenv.axon_hooks")
        mod._hook = hook
        mod.set_axon_ntff_profile_hook = lambda h: setattr(mod, "_hook", h)
        mod.get_axon_ntff_profile_hook = lambda: mod._hook
        sys.modules["antenv.axon_hooks"] = mod
        antenv.axon_hooks = mod

        # Artifact upload needs bucket creds this container may not have;
        # keep trace artifacts local instead.
        import concourse.bass_utils as _bu

        _bu.upload_artifacts = lambda tmpdir: tmpdir
    except Exception:
        pass


def _build(mm_dtype_name: str) -> bass.Bass:
    mmdt = getattr(mybir.dt, mm_dtype_name)
    nc = bacc.Bacc(None)

    # consts layout along free dim: [wT 8*512 | xT 4*BL | mask KCH | bias 512]
    CW = 8 * H + 4 * BL + KCH + H
    enc = nc.declare_dram_parameter("enc", [BL, T, D], F32, isOutput=False)
    xrep = nc.declare_dram_parameter("xrep", [128, BL, D], F32, isOutput=False)
    xs = nc.declare_dram_parameter("xs", [BL, D], F32, isOutput=False)
    consts = nc.declare_dram_parameter("consts", [128, CW], F32, isOutput=False)
    out = nc.declare_dram_parameter("out", [BL, D], F32, isOutput=True)

    with tile.TileContext(nc) as tc, ExitStack() as ctx:
        const = ctx.enter_context(tc.tile_pool(name="const", bufs=1))
        encp = ctx.enter_context(tc.tile_pool(name="encp", bufs=4))
        work = ctx.enter_context(tc.tile_pool(name="work", bufs=2))
        ps_ctx = ctx.enter_context(tc.tile_pool(name="ps_ctx", bufs=2, space="PSUM"))
        ps_sm = ctx.enter_context(tc.tile_pool(name="ps_sm", bufs=3, space="PSUM"))
        ps_att = ctx.enter_context(tc.tile_pool(name="ps_att", bufs=1, space="PSUM"))

        # ---- constants / whole-kernel-lifetime tiles ----
        id128 = const.tile([128, 128], F32)
        make_identity(nc, id128)
        id1 = const.tile([1, 1], F32)
        nc.vector.memset(id1, 1.0)
        ones_row = const.tile([1, 128], F32)
        nc.vector.memset(ones_row, 1.0)
        ones_col = const.tile([128, 1], F32)
        nc.vector.memset(ones_col, 1.0)
        ones_b = const.tile([1, BL], F32)
        nc.vector.memset(ones_b, 1.0)

        consts_sb = const.tile([128, CW], F32)
        nc.sync.dma_start(consts_sb, consts[:])
        o = 0
        wT_sb = consts_sb[:, o : o + 8 * H].rearrange("p (c h) -> p c h", c=8)
        o += 8 * H
        xT_sb = consts_sb[:, o : o + 4 * BL].rearrange("p (c b) -> p c b", c=4)
        o += 4 * BL
        mask_sb = consts_sb[:, o : o + KCH]
        o += KCH
        bias_sb = consts_sb[0:1, o : o + H]

        xs_sb = const.tile([BL, D], F32)
        nc.sync.dma_start(xs_sb, xs[:])
        xrep_sb = const.tile([128, BL, D], F32)
        nc.sync.dma_start(xrep_sb, xrep[:])

        # context^T columns for the final dense, filled one batch at a time
        ctxT_sb = const.tile([128, 4, BL], mmdt)

        for b in range(BL):
            src = enc[b].rearrange("(p k) d -> p k d", p=128)
            halves = []
            for h in range(2):
                eh = encp.tile([128, KCH // 2, D], F32, tag="enc", name=f"enc_{b}_{h}")
                nc.sync.dma_start(eh, src[:, h * 8 : (h + 1) * 8, :])
                halves.append(eh)

            # scores[p, k] = sum_d enc[t(p,k), d] * x[b, d], then + mask[p, k]
            scores = work.tile([128, KCH], F32, tag="scores", name=f"scores_{b}")
            dummy = work.tile([128, 1], F32, tag="dummy", name=f"dummy_{b}")
            for k in range(KCH):
                nc.vector.scalar_tensor_tensor(
                    out=dummy.broadcast_to((128, D)),
                    in0=halves[k // 8][:, k % 8, :],
                    scalar=1.0,
                    in1=xrep_sb[:, b, :],
                    op0=AO.mult,
                    op1=AO.mult,
                    accum_out=scores[:, k : k + 1],
                )
            nc.vector.tensor_add(scores, scores, mask_sb)

            # global max over all 2048 scores
            m1 = work.tile([128, 1], F32, tag="m1", name=f"m1_{b}")
            nc.vector.tensor_reduce(out=m1, in_=scores, axis=mybir.AxisListType.X, op=AO.max)
            mT_ps = ps_sm.tile([1, 128], F32, tag="small", name=f"mT_{b}")
            nc.tensor.transpose(mT_ps, m1, id128)
            mneg = work.tile([1, 1], F32, tag="mneg", name=f"mneg_{b}")
            nc.vector.tensor_reduce(
                out=mneg, in_=mT_ps, axis=mybir.AxisListType.X, op=AO.max, negate=True
            )
            negm_ps = ps_sm.tile([128, 1], F32, tag="small", name=f"negm_ps_{b}")
            nc.tensor.matmul(negm_ps, lhsT=ones_row, rhs=mneg)
            negm_sb = work.tile([128, 1], F32, tag="negm_sb", name=f"negm_sb_{b}")
            nc.scalar.copy(negm_sb, negm_ps)

            # exp(scores - m), with per-partition partial sums as a side output
            expv = work.tile([128, KCH], mmdt, tag="expv", name=f"expv_{b}")
            s1 = work.tile([128, 1], F32, tag="s1", name=f"s1_{b}")
            nc.scalar.activation(
                out=expv, in_=scores, func=AF.Exp, bias=negm_sb, scale=1.0, accum_out=s1
            )
            s_ps = ps_sm.tile([1, 1], F32, tag="small", name=f"s_ps_{b}")
            nc.tensor.matmul(s_ps, lhsT=s1, rhs=ones_col)
            rs = work.tile([1, 1], F32, tag="rs", name=f"rs_{b}")
            nc.vector.reciprocal(rs, s_ps)

            # unnormalized context: ctx[1, d] = sum_t exp[t] * enc[t, d]
            ctx_ps = ps_ctx.tile([1, D], F32, tag="ctx", name=f"ctx_{b}")
            enc_mm = [eh.bitcast(mmdt) if mm_dtype_name != "float32" else eh for eh in halves]
            for k in range(KCH):
                nc.tensor.matmul(
                    ctx_ps,
                    lhsT=expv[:, k : k + 1],
                    rhs=enc_mm[k // 8][:, k % 8, :],
                    start=(k == 0),
                    stop=(k == KCH - 1),
                )

            # normalize by 1/sum while copying out of PSUM
            ctxn = work.tile([1, D], mmdt, tag="ctxn", name=f"ctxn_{b}")
            nc.scalar.activation(out=ctxn, in_=ctx_ps, func=AF.Copy, bias=0.0, scale=rs)

            # transpose [1, 512] -> 4 x [128, 1] columns for the dense lhsT
            ctxT_ps = ps_sm.tile([128, 4], F32, tag="small", name=f"ctxT_ps_{b}")
            for c in range(4):
                nc.tensor.transpose(
                    ctxT_ps[:, c : c + 1], ctxn[:, c * 128 : (c + 1) * 128], id1
                )
            nc.scalar.copy(ctxT_sb[:, :, b], ctxT_ps)

        # ---- final dense over all local batches ----
        att_ps = ps_att.tile([BL, H], F32)
        wT_mm = wT_sb.bitcast(mmdt) if mm_dtype_name != "float32" else wT_sb
        xT_mm = xT_sb.bitcast(mmdt) if mm_dtype_name != "float32" else xT_sb
        for c in range(4):
            nc.tensor.matmul(
                att_ps, lhsT=xT_mm[:, c, :], rhs=wT_mm[:, c, :], start=(c == 0), stop=False
            )
        for c in range(4):
            nc.tensor.matmul(
                att_ps, lhsT=ctxT_sb[:, c, :], rhs=wT_mm[:, 4 + c, :], start=False, stop=False
            )
        nc.tensor.matmul(att_ps, lhsT=ones_b, rhs=bias_sb, start=False, stop=True)

        att_sb = work.tile([BL, H], F32, tag="att")
        nc.scalar.activation(att_sb, att_ps, AF.Sigmoid)
        res = work.tile([BL, D], F32, tag="res")
        nc.vector.tensor_mul(res, att_sb, xs_sb)
        nc.sync.dma_start(out[:], res)

    nc.finalize()
    return nc


def _get_nc() -> bass.Bass:
    key = MM_DTYPE
    if key not in _CACHE:
        _CACHE[key] = _build(key)
    return _CACHE[key]


LAST_RESULTS = None  # BassKernelResults of the most recent run (for test harness)


def kernel(x, enc_outs, W, b, actual_step, trace: bool = False) -> np.ndarray:
    x = np.ascontiguousarray(np.asarray(x, dtype=np.float32))
    enc = np.ascontiguousarray(np.asarray(enc_outs, dtype=np.float32))
    W = np.ascontiguousarray(np.asarray(W, dtype=np.float32))
    bvec = np.ascontiguousarray(np.asarray(b, dtype=np.float32)).reshape(1, H)
    step = int(np.asarray(actual_step))

    maskv = np.zeros(T, dtype=np.float32)
    if 0 <= step < T:
        maskv[step] = NEG
    mask2d = maskv.reshape(128, KCH)
    wTr = W.reshape(8, 128, H).transpose(1, 0, 2).reshape(128, 8 * H)
    bias_blk = np.zeros((128, H), np.float32)
    bias_blk[0] = bvec[0]

    in_maps = []
    for i in range(N_CORES):
        xs_i = x[i * BL : (i + 1) * BL]
        enc_i = enc[i * BL : (i + 1) * BL]
        xrep_i = np.ascontiguousarray(np.broadcast_to(xs_i[None], (128, BL, D)))
        xT_i = xs_i.T.reshape(4, 128, BL).transpose(1, 0, 2).reshape(128, 4 * BL)
        consts_i = np.ascontiguousarray(
            np.concatenate([wTr, xT_i, mask2d, bias_blk], axis=1)
        )
        in_maps.append(
            {
                "enc": enc_i,
                "xrep": xrep_i,
                "xs": np.ascontiguousarray(xs_i),
                "consts": consts_i,
            }
        )

    nc = _get_nc()
    if trace:
        _ensure_ntff_hook()
    res = run_bass_kernel_spmd(nc, in_maps, core_ids=list(range(N_CORES)), trace=trace)
    global LAST_RESULTS
    LAST_RESULTS = res
    return np.concatenate([res.results[i]["out"] for i in range(N_CORES)], axis=0)
